# revision 2
# baseline (speedup 1.0000x reference)
"""AdaFace loss on 8 TRN2 NeuronCores, class-parallel.

Strategy: shard the 100k weight rows (classes) across 8 cores. Host
pre-normalizes rows, transposes to [D, C_shard], scales by 8 and casts to
fp8e4 (scale keeps values out of the e4m3 subnormal range; the exp
absorbs it: exp(0.5*x - 32) of the 64*cos matmul result). Since
|logit| <= 32, a fixed shift of 32 replaces the per-row max of a
standard log-softmax, so no max collective is needed. Each core returns
per-batch-chunk partial sums of exp(32c-32); the host does the final
O(B) combine: sum across cores, margin-target correction, ln, weighted
dot. No device collective.

Device pipeline (v2): the PSUM-drain is the bottleneck, so both
PSUM-capable engines run self-contained softmax-denominator pipelines:
 - ScalarE (ACT): exp activation with fused accum_out row-sum
   (one instruction per class tile, ~2.27us per 2048 cols).
 - VectorE (DVE): Schraudolph fake-exp (affine to i16 whose bits are
   bf16(exp)), bf16 adds into eacc, one tensor_reduce per batch chunk.
ACT takes ~15 of 24 wide tiles + all 256-col tails; DVE takes ~9.
The PE streams 512-col fp8 DoubleRow matmuls into two rotating 4-bank
PSUM tiles; consumers alternate so the PE never waits long. All partial
sums land in one [128, NSLOT] f32 tile, DMA'd out once; the host sums
slots, applies the margin-target correction and the weighted log.
"""

import numpy as np
import ml_dtypes

import concourse.bass as bass
import concourse.tile as tile
from concourse import bacc, mybir
from concourse.bass_utils import run_bass_kernel_spmd

B = 512
D = 256
C = 100000
NCORES = 8
CSH = C // NCORES          # 12500 classes per core
CPAD = 12544               # 6*2048 + 256
NPAD_TOT = (CPAD - CSH) * NCORES

M0 = 0.5
M_MIN = 0.25
SCALE = 32.0
SHIFT = 32.0               # fixed log-softmax shift (|logits| <= SCALE)
FP8_PRESCALE = 8.0         # both operands scaled by 8 -> matmul gives 64*cos

# Schraudolph fake-exp: from x = 64*cos, bf16 bits of exp(0.5*x - 32)
# are i16 = rint(x*FA + FB_EFF); HW rounds to nearest on f32->i16
LOG2E = 1.4426950408889634
FA = 64.0 * LOG2E
FB = 16256.0 - 4096.0 * LOG2E

f32 = mybir.dt.float32
bf16 = mybir.dt.bfloat16
i16 = mybir.dt.int16
fp8 = mybir.dt.float8e4

NBC = B // 128             # 4 batch chunks

# Per-chunk tile assignment: 6 wide (2048) tiles T0..T5 + small (256) T6.
# "A" tiles -> ACT exp+accum; "D" tiles -> DVE fake-exp.
# 3 chunks with 4A/2D + 1 chunk with 3A/3D: ACT 15 wide + 4 small,
# DVE 9 wide.  (ACT ~37.0us, DVE ~36.4us projected.)
CHUNK_NDVE = [2, 2, 2, 3]

_cached_nc = None
_last_results = None


def _schraudolph_rho(fb):
    t = np.linspace(-60.0, -1.0, 200001)
    x = (t + 32.0) * 2.0
    y = np.float32(x) * np.float32(FA) + np.float32(fb)
    i = np.rint(y).astype(np.int16)
    v = i.view(ml_dtypes.bfloat16).astype(np.float64)
    return float(np.mean(v / np.exp(t)))


# fold the mean fake/real ratio into the offset (de-bias)
FB_EFF = FB - 128.0 * np.log2(_schraudolph_rho(FB))
FB_EFF = FB_EFF - 128.0 * np.log2(_schraudolph_rho(FB_EFF))


def _build():
    global _cached_nc
    if _cached_nc is not None:
        return _cached_nc

    nc = bacc.Bacc(
        "TRN2", target_bir_lowering=False, debug=False, num_devices=NCORES
    )

    # [p, j, c] with contraction index k = j*128 + p
    wnT_d = nc.dram_tensor("wnT", [128, 2, CPAD], fp8, kind="ExternalInput")
    featnT_d = nc.dram_tensor("featnT", [128, 2, B], fp8, kind="ExternalInput")
    # slots: per chunk 5 ACT slots (4 wide + small; some unused) and
    # 1 DVE slot -> [128, NBC, 6]
    NSLOT = 6
    out_d = nc.dram_tensor("out", [128, NBC, NSLOT], f32, kind="ExternalOutput")

    with tile.TileContext(nc) as tc:
        with (
            tc.tile_pool(name="persist", bufs=1) as persist,
            tc.tile_pool(name="work", bufs=2) as work,
            tc.tile_pool(name="psum", bufs=2, space="PSUM") as psum,
        ):
            fsb = persist.tile([128, 2, B], fp8)
            nc.sync.dma_start(out=fsb[:], in_=featnT_d[:])

            wsb = persist.tile([128, 2, CPAD], fp8)
            # chunked weight loads in consumption order across two HWDGE
            # queues (a single queue serializes; gpsimd SWDGE is too slow)
            plan = [
                (nc.scalar, 0, 1536),
                (nc.sync, 1536, 3072),
                (nc.scalar, 3072, 4608),
                (nc.sync, 4608, 6144),
                (nc.scalar, 6144, 7680),
                (nc.sync, 7680, 9216),
                (nc.scalar, 9216, 10752),
                (nc.sync, 10752, 12288),
                (nc.scalar, 12288, 12544),
            ]
            for eng, lo, hi in plan:
                eng.dma_start(out=wsb[:, :, lo:hi], in_=wnT_d[:, :, lo:hi])

            bias_s = persist.tile([128, 1], f32)
            nc.gpsimd.memset(bias_s[:], -SHIFT)

            S_out = persist.tile([128, NBC, NSLOT], f32)
            nc.gpsimd.memset(S_out[:], 0.0)

            esc = persist.tile([128, 2048], bf16)   # ACT dead-store target

            for bc in range(NBC):
                nd = CHUNK_NDVE[bc]
                lhs = fsb[:, :, bc * 128:(bc + 1) * 128]
                # interleave: A D A D A (D|A) small
                order = []
                a_slot = 0
                dve_tiles = []
                for ti in range(6):
                    is_dve = (ti % 2 == 1 and len(dve_tiles) < nd) or \
                             (ti >= 4 and len(dve_tiles) < nd and
                              6 - ti <= nd - len(dve_tiles))
                    order.append((ti, is_dve))
                    if is_dve:
                        dve_tiles.append(ti)

                eacc = work.tile([128, 2048], bf16, tag="eacc")
                ndone = 0
                for ti, is_dve in order:
                    c0 = ti * 2048
                    ps = psum.tile([128, 2048], f32, tag="ps")
                    for j in range(0, 2048, 512):
                        nc.tensor.matmul(
                            ps[:, j:j + 512],
                            lhs,
                            wsb[:, :, c0 + j:c0 + j + 512],
                            start=True, stop=True,
                            perf_mode=mybir.MatmulPerfMode.DoubleRow,
                        )
                    if not is_dve:
                        nc.scalar.activation(
                            esc[:], ps[:],
                            mybir.ActivationFunctionType.Exp,
                            bias=bias_s[:], scale=SCALE / (FP8_PRESCALE**2),
                            accum_out=S_out[:, bc, a_slot:a_slot + 1],
                        )
                        a_slot += 1
                    else:
                        fi = work.tile([128, 2048], i16, tag="fi")
                        nc.vector.tensor_scalar(
                            fi[:], ps[:],
                            FA, FB_EFF,
                            mybir.AluOpType.mult, mybir.AluOpType.add,
                        )
                        ndone += 1
                        if ndone == 1:
                            fi_first = fi
                        elif ndone == 2:
                            nc.vector.tensor_add(
                                eacc[:], fi_first[:].bitcast(bf16),
                                fi[:].bitcast(bf16),
                            )
                        else:
                            nc.vector.tensor_add(
                                eacc[:], eacc[:], fi[:].bitcast(bf16)
                            )

                # small 256-col tail tile -> ACT
                ps = psum.tile([128, 2048], f32, tag="ps")
                nc.tensor.matmul(
                    ps[:, 0:256],
                    lhs,
                    wsb[:, :, 12288:12544],
                    start=True, stop=True,
                    perf_mode=mybir.MatmulPerfMode.DoubleRow,
                )
                nc.scalar.activation(
                    esc[:, 0:256], ps[:, 0:256],
                    mybir.ActivationFunctionType.Exp,
                    bias=bias_s[:], scale=SCALE / (FP8_PRESCALE**2),
                    accum_out=S_out[:, bc, a_slot:a_slot + 1],
                )

                # chunk row-sum of the DVE accumulator
                nc.vector.tensor_reduce(
                    S_out[:, bc, NSLOT - 1:NSLOT],
                    eacc[:],
                    axis=mybir.AxisListType.X,
                    op=mybir.AluOpType.add,
                )

            nc.sync.dma_start(out=out_d[:], in_=S_out[:])

    nc.compile()
    _cached_nc = nc
    return nc


def _host_prep(features, weight, weights, labels):
    """Everything O(B*D) / O(C*D) that is not the big matmul."""
    f = features.astype(np.float64)
    norms = np.sqrt((f * f).sum(axis=1))
    lo, hi = norms.min(), norms.max()
    denom = max(hi - lo, 1e-8)
    margins = np.clip(M_MIN + (M0 - M_MIN) * (norms - lo) / denom, M_MIN, M0)
    feat_n = f / np.maximum(norms, 1e-12)[:, None]

    wlab = weight[labels].astype(np.float64)
    wlab_n = wlab / np.maximum(
        np.sqrt((wlab * wlab).sum(axis=1)), 1e-12
    )[:, None]
    cos_t = np.clip((feat_n * wlab_n).sum(axis=1), -1.0 + 1e-7, 1.0 - 1e-7)
    cos_m = cos_t * np.cos(margins) - np.sqrt(1.0 - cos_t * cos_t) * np.sin(
        margins
    )
    t_logit = SCALE * cos_m
    corr = (
        np.exp(SCALE * cos_m - SHIFT)
        - np.exp(SCALE * cos_t - SHIFT)
        - NPAD_TOT * np.exp(-SHIFT)
    )
    coef = weights.astype(np.float64) / B
    return feat_n, corr, coef, t_logit


def _to_dr_layout(mat_t, width):
    """[D, X] f32 -> [128, 2, X] fp8 with k = j*128 + p."""
    a = mat_t.reshape(2, 128, width)          # [j, p, X]
    a = np.ascontiguousarray(a.transpose(1, 0, 2))  # [p, j, X]
    return a.astype(ml_dtypes.float8_e4m3)


def kernel(features, weight, weights, labels):
    global _last_results
    features = np.asarray(features, dtype=np.float32)
    weight = np.asarray(weight, dtype=np.float32)
    weights = np.asarray(weights, dtype=np.float32)
    labels = np.asarray(labels).astype(np.int64)

    feat_n, corr, coef, t_logit = _host_prep(features, weight, weights, labels)

    wn = weight / np.maximum(
        np.linalg.norm(weight, axis=1, keepdims=True), 1e-12
    )
    featnT = np.ascontiguousarray(feat_n.T.astype(np.float32)) * FP8_PRESCALE
    featnT8 = _to_dr_layout(featnT, B)

    in_maps = []
    for i in range(NCORES):
        sh = wn[i * CSH:(i + 1) * CSH]  # [CSH, D]
        wt = np.zeros((D, CPAD), dtype=np.float32)
        wt[:, :CSH] = sh.T * FP8_PRESCALE
        in_maps.append(
            {"wnT": _to_dr_layout(wt, CPAD), "featnT": featnT8}
        )

    nc = _build()
    res = run_bass_kernel_spmd(nc, in_maps, list(range(NCORES)))
    _last_results = res

    # ---- host combine ----
    S = np.zeros(B, dtype=np.float64)
    for i in range(NCORES):
        sc = np.asarray(res.results[i]["out"], dtype=np.float64)  # [128,NBC,6]
        for bc in range(NBC):
            S[bc * 128:(bc + 1) * 128] += sc[:, bc, :].sum(axis=1)

    Z = S + corr
    per = SHIFT + np.log(Z) - t_logit
    loss = float((coef * per).sum())
    return np.array(loss, dtype=np.float32)
